# revision 35
# baseline (speedup 1.0000x reference)
"""Trainium2 Bass kernel for nn_NisuyNN_90434831384984.

Math: the reference's stack+reshape makes MLP row (s,t,b) depend only on s
(b in {0,1}) or only on t (b in {2,3}), and rows for b=2,3 equal those for
b=0,1 — so the 4096-row x 6-layer MLP collapses to 64 unique rows producing
64 unique 32x32 policy matrices.  The 50-step power iteration has a large
eigengap; 3 unnormalized extra steps after the row-sum init reach the bf16
noise floor (the final deltas use only intra-vector ratios, so scale
cancels).

Distribution: layer 1 is fully replicated on all 8 cores (W1 is only 2MB;
this removes the first AllGather so the one-time ~60us collectives-init
barrier hides behind layers 1-2 compute).  Layers 2-5 are Megatron
column-split with a per-layer AllGather of the locally transposed
activation slice.  Layer 6 keeps the full W6 on every core (policy matrix
fully local; no collective), then the power-iteration + deltas tail runs
replicated and core 0's output is returned.

Weights are host-prelayouted chunk-major ([128, nk*width], chunk = 128
contiguous K rows) so every weight DMA moves 8KB-contiguous lines per
partition instead of 1KB gather segments.
"""

import numpy as np

DIM = 128
N = 32
B = 4
H = 4096
NC = 8          # cores
SL = H // NC    # 512 hidden slice per core
OF = N * N      # 1024 output features
R = 64          # unique MLP rows
KC = 128        # contraction chunk
TK = 32         # K-chunks per weight tile (whole layer in one tile)
PI_ITERS = 2    # extra matvec iterations after the init row-sum step
SLOPE = 0.01
WSCALE = 16.0   # weights are shipped as fp8e4m3 scaled by 16 (keeps them
                # out of the subnormal range); descale inside the Lrelu

_COMPILED = None
LAST_RESULTS = None


def _build_body(nc, tc, tile, mybir, aps):
    f32 = mybir.dt.float32
    bf16 = mybir.dt.bfloat16
    f8 = mybir.dt.float8e4
    AF = mybir.ActivationFunctionType
    ALU = mybir.AluOpType
    AX = mybir.AxisListType
    rg = [list(range(NC))]

    from contextlib import ExitStack
    es = ExitStack()
    cpool = es.enter_context(tc.tile_pool(name="consts", bufs=1))
    wpool = es.enter_context(tc.tile_pool(name="w", bufs=5))
    bpool = es.enter_context(tc.tile_pool(name="b", bufs=2))
    apool = es.enter_context(tc.tile_pool(name="act", bufs=2))
    atp = es.enter_context(tc.tile_pool(name="atT", bufs=2))
    lpool = es.enter_context(tc.tile_pool(name="lhs", bufs=3))
    pipool = es.enter_context(tc.tile_pool(name="pi", bufs=2))
    tailp = es.enter_context(tc.tile_pool(name="tail", bufs=1))
    ps = es.enter_context(tc.tile_pool(name="ps", bufs=3, space="PSUM"))
    pst = es.enter_context(tc.tile_pool(name="pst", bufs=4, space="PSUM"))
    dram = es.enter_context(tc.tile_pool(name="dram", bufs=3, space="DRAM"))

    # Warm up the collective path first: the first collective pays a ~50us
    # one-time init barrier; trigger it immediately (input is host-provided
    # so the doorbell fires as soon as the gpsimd preamble ends) and absorb
    # it behind layers 1-2, which need no collective.
    warm_in = dram.tile([KC, 8], bf16, tag="warm_in")
    warm_out = dram.tile([NC * KC, 8], bf16, tag="warm_out", addr_space="Shared")
    # warm_in is deliberately never written: the gathered bytes are ignored,
    # and skipping the staging DMA lets the doorbell (and with it the global
    # CC-init barrier that all cores must join) fire as early as possible.
    nc.gpsimd.collective_compute(
        "AllGather", ALU.bypass, replica_groups=rg,
        ins=[warm_in[:].opt()], outs=[warm_out[:].opt()],
    )

    # ---- constants ----
    id64 = cpool.tile([64, 64], bf16)
    nc.gpsimd.dma_start(id64[:], aps["ID64"][:])
    dmask = cpool.tile([R, N], f32)
    nc.gpsimd.dma_start(dmask[:], aps["DMASK"][:])
    t01 = cpool.tile([R, 1], f32)  # host-precomputed row sums of T[0],T[1]
    nc.gpsimd.dma_start(t01[:], aps["T01"][:])
    tt23 = cpool.tile([R, N], f32)
    nc.gpsimd.dma_start(tt23[:], aps["TT23"][:])
    mac = cpool.tile([R, 2], f32)
    nc.gpsimd.dma_start(mac[:], aps["MAC"][:])
    ones = cpool.tile([1, R], bf16)
    nc.vector.memset(ones[:], 1.0)
    # keep the warm AG live via a tiny output DMA (gpsimd, like baseline)
    nc.gpsimd.dma_start(aps["warm"][:], warm_out[0:1, :])

    def load_w(w_ap, nk, ncols):
        """Stream [128, nk*ncols] chunk-major fp8 weights as contiguous
        tiles of TK chunks (one whole layer: 16KB per partition line)."""
        wtiles = []
        for t in range(0, nk, TK):
            n = min(TK, nk - t)
            wt = wpool.tile([KC, n * ncols], f8, tag="w")
            nc.sync.dma_start(wt[:], w_ap[:, t * ncols:(t + n) * ncols])
            wtiles.append(wt)
        return wtiles

    def mm_layer(pt, lhs_of, wt_of, nk, btile, bofs, bw):
        """Accumulate psum [128, bw]: rows 0:64 = bias + even K-chunks,
        rows 64:128 = odd K-chunks; the two PE column groups pipeline.
        Bias goes first so the even half finishes (and the h0 copy can
        start) before the final odd-half matmul completes."""
        nc.tensor.matmul(
            pt[0:R, :], ones[0:1, :], btile[0:1, bofs:bofs + bw],
            start=True, stop=False, tile_position=(0, 0),
            skip_group_check=True,
        )
        first_h1 = True
        n_by_half = [0, 0]
        for k in range(nk):
            h = k % 2
            n_by_half[h] += 1
            is_last = n_by_half[h] == (nk + 1 - h) // 2
            nc.tensor.matmul(
                pt[h * R:(h + 1) * R, :],
                lhs_of(k),
                wt_of(k),
                start=(h == 1 and first_h1),
                stop=is_last,
                tile_position=(0, h * R),
                skip_group_check=True,
            )
            if h == 1:
                first_h1 = False

    def sum_halves(pt, hw):
        """z = psum_rows0 + psum_rows64 (DVE can read only one PSUM input)."""
        h0 = apool.tile([R, hw], f32, tag="h0")
        nc.scalar.activation(h0[:], pt[0:R, :], AF.Copy)
        z = apool.tile([R, hw], f32, tag="zsum")
        nc.vector.tensor_tensor(z[:], h0[:], pt[R:2 * R, :], op=ALU.add)
        return z

    def lrelu_act(pt, hw):
        """act_bf16 = leaky_relu((psum_rows0 + psum_rows64) / WSCALE)."""
        z = sum_halves(pt, hw)
        act = apool.tile([R, hw], bf16, tag="act")
        nc.scalar.activation(act[:], z[:], AF.Lrelu, alpha=SLOPE,
                             scale=1.0 / WSCALE)
        return act

    # ---- layer 1, fully replicated: [64, 256] @ [256, 4096] ----
    xt = lpool.tile([KC, 2 * R], bf16, tag="xt", bufs=1)
    nc.sync.dma_start(xt[:], aps["XT"][:])
    w1t = wpool.tile([KC, 2 * H], f8, tag="w1", bufs=1)  # both K-chunks
    nc.sync.dma_start(w1t[:], aps["W1"][:])
    b1t = bpool.tile([1, H], bf16, tag="b1", bufs=1)
    nc.sync.dma_start(b1t[0:1, :], aps["b1"].unsqueeze(0))

    lt = lpool.tile([KC, H // KC * R], bf16, tag="lhs")  # [128, 2048]
    for blk in range(8):
        pt = ps.tile([2 * R, SL], f32, tag="ps")
        mm_layer(
            pt,
            lambda k: xt[:, k * R:(k + 1) * R],
            lambda k: w1t[:, k * H + blk * SL:k * H + (blk + 1) * SL],
            2, b1t, blk * SL, SL,
        )
        act = lrelu_act(pt, SL)
        for j in range(4):
            tp = pst.tile([KC, R], bf16, tag="pst")
            nc.tensor.transpose(tp[:], act[:, j * KC:(j + 1) * KC], id64[:])
            dst = lt[:, (blk * 4 + j) * R:(blk * 4 + j + 1) * R]
            if j % 2 == 0:
                nc.vector.tensor_copy(dst, tp[:])
            else:
                nc.scalar.activation(dst, tp[:], AF.Copy)

    # ---- layers 2-5: Megatron column split, AllGather per layer ----
    for li in range(2, 6):
        nk = H // KC
        wtiles = load_w(aps[f"W{li}"], nk, SL)
        btile = bpool.tile([1, SL], bf16, tag="bias")
        nc.sync.dma_start(btile[0:1, :], aps[f"b{li}"].unsqueeze(0))
        pt = ps.tile([2 * R, SL], f32, tag="ps")
        mm_layer(
            pt,
            lambda k: lt[:, k * R:(k + 1) * R],
            lambda k: wtiles[k // TK][:, (k % TK) * SL:(k % TK + 1) * SL],
            nk, btile, 0, SL,
        )
        act = lrelu_act(pt, SL)
        # transpose the [64, 512] slice to [128, 4*64] and AllGather it
        att = atp.tile([KC, 4 * R], bf16, tag="atT")
        for j in range(4):
            tp = pst.tile([KC, R], bf16, tag="pst")
            nc.tensor.transpose(tp[:], act[:, j * KC:(j + 1) * KC], id64[:])
            dst = att[:, j * R:(j + 1) * R]
            if j % 2 == 0:
                nc.vector.tensor_copy(dst, tp[:])
            else:
                nc.scalar.activation(dst, tp[:], AF.Copy)
        ag_in = dram.tile([KC, 4 * R], bf16, tag="agin")
        nc.scalar.dma_start(ag_in[:], att[:])
        ag_out = dram.tile([NC * KC, 4 * R], bf16, tag="agout",
                           addr_space="Shared")
        nc.gpsimd.collective_compute(
            "AllGather", ALU.bypass, replica_groups=rg,
            ins=[ag_in[:].opt()], outs=[ag_out[:].opt()],
        )
        # rank-pair loads back to SBUF (2 x 512B segments per partition),
        # spread across queues
        lt = lpool.tile([KC, NC * 4 * R], bf16, tag="lhs")
        for r in range(0, NC, 2):
            src = ag_out[r * KC:(r + 2) * KC, :].rearrange(
                "(r p) c -> p r c", p=KC)
            dst = lt[:, r * 4 * R:(r + 2) * 4 * R].rearrange(
                "p (r c) -> p r c", r=2)
            eng = (nc.scalar, nc.sync, nc.gpsimd, nc.scalar)[r // 2]
            eng.dma_start(dst, src)

    # ---- layer 6: full W6 on every core (no collective), K=4096, N=1024 ----
    nk6 = H // KC
    b6t = bpool.tile([1, OF], bf16, tag="b6")
    nc.sync.dma_start(b6t[0:1, :], aps["b6"].unsqueeze(0))
    M = pipool.tile([R, OF], f32, tag="M")
    for nb in range(2):
        w6tiles = load_w(aps[f"W6{'ab'[nb]}"], nk6, SL)
        pt = ps.tile([2 * R, SL], f32, tag="ps")
        mm_layer(
            pt,
            lambda k: lt[:, k * R:(k + 1) * R],
            lambda k: w6tiles[k // TK][:, (k % TK) * SL:(k % TK + 1) * SL],
            nk6, b6t, nb * SL, SL,
        )
        z = sum_halves(pt, SL)
        lr = apool.tile([R, SL], f32, tag="lrelu_out")
        nc.scalar.activation(lr[:], z[:], AF.Lrelu, alpha=SLOPE,
                             scale=1.0 / WSCALE)
        sg = apool.tile([R, SL], f32, tag="sig")
        nc.scalar.activation(sg[:], lr[:], AF.Sigmoid)
        nc.vector.tensor_scalar(
            M[:, nb * SL:(nb + 1) * SL], sg[:], mac[:, 0:1], mac[:, 1:2],
            op0=ALU.mult, op1=ALU.add,
        )

    # ---- power iteration: b <- M b, unnormalized ----
    # each mult+segmented-reduce step is split between the Vector and GpSimd
    # engines (disjoint r-blocks) to halve the serial latency
    RS = 24  # rows 0:RS on vector, RS:32 on gpsimd (gpsimd is slower)
    M3 = M[:].rearrange("p (r q) -> p r q", q=N)
    Mb = pipool.tile([R, OF], bf16, tag="Mb")
    nc.scalar.activation(Mb[:], M[:], AF.Copy)
    Mb3 = Mb[:].rearrange("p (r q) -> p r q", q=N)
    bv = pipool.tile([R, N], f32, tag="bv")
    nc.vector.reduce_sum(bv[:], M3, axis=AX.X)  # first step from b0 = ones
    for it in range(PI_ITERS):
        last = it == PI_ITERS - 1
        bb = bv[:].unsqueeze(1).broadcast_to((R, N, N))
        if last:
            tmp = pipool.tile([R, OF], f32, tag="pit")
            t3 = tmp[:].rearrange("p (r q) -> p r q", q=N)
            src3 = M3
        else:
            tmp = pipool.tile([R, OF], bf16, tag="pitb")
            t3 = tmp[:].rearrange("p (r q) -> p r q", q=N)
            src3 = Mb3
        nc.vector.tensor_tensor(t3[:, 0:RS], src3[:, 0:RS], bb[:, 0:RS],
                                op=ALU.mult)
        nc.gpsimd.tensor_tensor(t3[:, RS:N], src3[:, RS:N], bb[:, RS:N],
                                op=ALU.mult)
        bv = pipool.tile([R, N], f32, tag="bv")
        nc.vector.reduce_sum(bv[:, 0:RS], t3[:, 0:RS], axis=AX.X)
        nc.vector.reduce_sum(bv[:, RS:N], t3[:, RS:N], axis=AX.X)

    # ---- deltas tail ----
    scr = tailp.tile([R, N], f32, tag="scr")
    d = tailp.tile([R, 1], f32, tag="d")
    nc.vector.tensor_tensor(scr[:], bv[:], dmask[:], op=ALU.mult)
    nc.vector.reduce_sum(d[:], scr[:], axis=AX.X)
    recipd = tailp.tile([R, 1], f32, tag="rd")
    nc.vector.reciprocal(recipd[:], d[:])
    recipE = tailp.tile([R, N], f32, tag="rE")
    nc.vector.reciprocal(recipE[:], bv[:])
    coef_s = tailp.tile([R, 1], f32, tag="cs")
    nc.vector.tensor_tensor(coef_s[:], t01[:, 0:1], recipd[:], op=ALU.mult)
    scr2 = tailp.tile([R, N], f32, tag="scr2")
    c23 = tailp.tile([R, 1], f32, tag="c23")
    nc.vector.tensor_tensor(scr2[:], tt23[:], recipE[:], op=ALU.mult)
    nc.vector.reduce_sum(c23[:], scr2[:], axis=AX.X)
    coef = tailp.tile([R, B], f32, tag="coef")
    nc.vector.memset(coef[:], 0.0)
    nc.vector.tensor_copy(coef[0:32, 0:1], coef_s[0:32, :])
    nc.vector.tensor_copy(coef[32:64, 1:2], coef_s[32:64, :])
    nc.vector.tensor_copy(coef[0:32, 2:3], c23[0:32, :])
    nc.vector.tensor_copy(coef[32:64, 3:4], c23[32:64, :])
    pd = pst.tile([B, N], f32, tag="pd", bufs=1)
    nc.tensor.matmul(pd[:], coef[:], bv[:], start=True, stop=True)
    osb = tailp.tile([B, N], f32, tag="osb")
    nc.vector.tensor_copy(osb[:], pd[:])
    nc.sync.dma_start(aps["out"][:], osb[:])
    es.close()


def build():
    import concourse.bacc as bacc
    import concourse.mybir as mybir
    import concourse.tile as tile

    f32 = mybir.dt.float32
    bf16 = mybir.dt.bfloat16
    nc = bacc.Bacc("TRN2", target_bir_lowering=False, debug=False, num_devices=NC)
    f8 = mybir.dt.float8e4
    shapes = {
        "XT": ([KC, 2 * R], bf16),
        "W1": ([KC, 2 * H], f8), "b1": ([H], bf16),
        "W2": ([KC, H // KC * SL], f8), "b2": ([SL], bf16),
        "W3": ([KC, H // KC * SL], f8), "b3": ([SL], bf16),
        "W4": ([KC, H // KC * SL], f8), "b4": ([SL], bf16),
        "W5": ([KC, H // KC * SL], f8), "b5": ([SL], bf16),
        "W6a": ([KC, H // KC * SL], f8), "W6b": ([KC, H // KC * SL], f8),
        "b6": ([OF], bf16),
        "T01": ([R, 1], f32), "TT23": ([R, N], f32),
        "DMASK": ([R, N], f32), "MAC": ([R, 2], f32), "ID64": ([64, 64], bf16),
        "WARMIN": ([KC, 8], bf16),
    }
    aps = {
        k: nc.dram_tensor(k, v[0], v[1], kind="ExternalInput").ap()
        for k, v in shapes.items()
    }
    aps["out"] = nc.dram_tensor("out", [B, N], f32, kind="ExternalOutput").ap()
    aps["warm"] = nc.dram_tensor("warm", [1, 8], bf16, kind="ExternalOutput").ap()
    with tile.TileContext(nc) as tc:
        _build_body(nc, tc, tile, mybir, aps)
    nc.compile()
    return nc


def _chunk_major(W):
    """[K, width] -> [128, (K//128)*width]; chunk k's rows land contiguous."""
    K, width = W.shape
    nk = K // KC
    return np.ascontiguousarray(
        W.reshape(nk, KC, width).transpose(1, 0, 2).reshape(KC, nk * width)
    )


def prep_in_maps(inputs):
    import ml_dtypes
    f = np.float32
    bf = ml_dtypes.bfloat16
    f8 = ml_dtypes.float8_e4m3fn
    E = np.asarray(inputs["batch_node_embeddings"], f)   # (B,N,D)
    T = np.asarray(inputs["batch_Ts"], f)                # (B,N,N)
    mult = np.asarray(inputs["mult_const_batch"], f).reshape(-1)[0]
    add = np.asarray(inputs["add_const_batch"], f).reshape(-1)[0]
    S = np.transpose(E, (1, 0, 2))                       # (N,B,D)
    G0 = np.concatenate([S[:, 0], S[:, 1]], axis=-1)     # (N, 2D)
    G1 = np.concatenate([S[:, 2], S[:, 3]], axis=-1)
    rows = np.concatenate([G0, G1], axis=0)              # (64, 256)
    XT = np.ascontiguousarray(rows.T)                    # (256, 64)
    W6 = np.asarray(inputs["W6"], f)
    common = {
        "XT": _chunk_major(XT).astype(bf),
        "W1": _chunk_major(np.asarray(inputs["W1"], f) * WSCALE).astype(f8),
        "b1": (np.asarray(inputs["b1"], f) * WSCALE).astype(bf),
        "W6a": _chunk_major(W6[:, 0:512] * WSCALE).astype(f8),
        "W6b": _chunk_major(W6[:, 512:1024] * WSCALE).astype(f8),
        "b6": (np.asarray(inputs["b6"], f) * WSCALE).astype(bf),
        "T01": np.ascontiguousarray(
            np.concatenate([T[0], T[1]], axis=0).sum(axis=1, keepdims=True)
        ),
        "TT23": np.ascontiguousarray(np.concatenate([T[2].T, T[3].T], axis=0)),
        "DMASK": np.ascontiguousarray(np.tile(np.eye(N, dtype=f), (2, 1))),
        "MAC": np.ascontiguousarray(
            np.stack([np.full(R, mult, f), np.full(R, add, f)], axis=1)
        ),
        "ID64": np.eye(64, dtype=bf),
        "WARMIN": np.zeros((KC, 8), bf),
    }
    in_maps = []
    for c in range(NC):
        m = dict(common)
        for li in range(2, 6):
            W = np.asarray(inputs[f"W{li}"], f)
            b = np.asarray(inputs[f"b{li}"], f)
            m[f"W{li}"] = _chunk_major(
                W[:, c * SL:(c + 1) * SL] * WSCALE).astype(f8)
            m[f"b{li}"] = np.ascontiguousarray(
                b[c * SL:(c + 1) * SL] * WSCALE).astype(bf)
        in_maps.append(m)
    return in_maps


def kernel(**inputs):
    global _COMPILED, LAST_RESULTS
    from concourse import bass_utils

    if _COMPILED is None:
        _COMPILED = build()
    in_maps = prep_in_maps(inputs)
    res = bass_utils.run_bass_kernel_spmd(
        _COMPILED, in_maps, core_ids=list(range(NC))
    )
    LAST_RESULTS = res
    return np.asarray(res.results[0]["out"], np.float32)
